# revision 2
# baseline (speedup 1.0000x reference)
"""Trainium2 Bass kernel for multi-head attention (B=4, F=2048, D=1024, H=16, dh=64).

Sharding: 8 cores = (batch b, q-half) — core c handles batch c//2, query rows
[ (c%2)*1024, (c%2+1)*1024 ) of that batch.  Each core computes the K/V
projections for its whole batch (duplicated across the 2 cores of a batch),
the Q projection for its own rows, all 16 heads of attention for its rows,
and the output projection.  Output row blocks are disjoint, so the host
simply concatenates per-core outputs — no inter-core communication.

Layout strategy (everything keeps the contraction dim on SBUF partitions):
 - Host pre-transposes activations: xqT/xkT/xvT are [1024(in), rows].
 - Projections produce qhT/khT transposed [head*64+d, rows] (lhsT = weight
   chunks) and vh natural [kv, head*64+d] (lhsT = xvT chunks).
 - S^T[kv, q] = khT_slice.T @ qhT_slice per (head, q-block, kv-tile); exp on
   ScalarE straight out of PSUM (scale 1/8 and q-bias folded into qhT).
 - PV: lhsT = [V | ones] [128kv, 65] so PSUM row 64 accumulates the softmax
   denominators; rhs = P^T.  Output O^T[d, q] normalized on the way to SBUF.
 - v-bias is added to vh, which after normalization contributes exactly +b.
 - Output projection: lhsT = O^T chunks, rhs = out_kernel [hd, m].

Compute dtype: bf16 operands, fp32 PSUM accumulation.
"""

import os
import sys
import types

sys.path.insert(0, "/opt/trn_rl_repo")

import numpy as np
import ml_dtypes

BF16_NP = ml_dtypes.bfloat16

B, F, D = 4, 2048, 1024
NH, DH = 16, 64
NQ = 1024          # q rows per core
NCORES = 8


def _install_ntff_hook_shim():
    """The agent image's antenv stub lacks axon_hooks; recreate it so
    run_bass_kernel_spmd(trace=True) can capture NTFF profiles."""
    if "antenv.axon_hooks" in sys.modules:
        return
    m = types.ModuleType("antenv.axon_hooks")
    m._hook = None

    def set_axon_ntff_profile_hook(h):
        m._hook = h

    def get_axon_ntff_profile_hook():
        return m._hook

    m.set_axon_ntff_profile_hook = set_axon_ntff_profile_hook
    m.get_axon_ntff_profile_hook = get_axon_ntff_profile_hook
    sys.modules["antenv.axon_hooks"] = m
    import antenv

    antenv.axon_hooks = m
    try:
        from trn_agent_boot.trn_boot import _ntff_profile_via_ctypes

        m._hook = _ntff_profile_via_ctypes("/opt/axon/libaxon_pjrt.so")
    except Exception:
        pass


_install_ntff_hook_shim()

import concourse.bass as bass
import concourse.bacc as bacc
import concourse.mybir as mybir
import concourse.tile as tile
from concourse import bass_utils

BF16 = mybir.dt.bfloat16
F32 = mybir.dt.float32
AF = mybir.ActivationFunctionType


def build_kernel():
    nc = bacc.Bacc("TRN2", target_bir_lowering=False, debug=False, num_devices=NCORES)

    xqT = nc.declare_dram_parameter("xqT", [D, NQ], BF16, isOutput=False)
    xkT = nc.declare_dram_parameter("xkT", [D, F], BF16, isOutput=False)
    xvT = nc.declare_dram_parameter("xvT", [D, F], BF16, isOutput=False)
    wq = nc.declare_dram_parameter("wq", [D, D], BF16, isOutput=False)
    wk = nc.declare_dram_parameter("wk", [D, D], BF16, isOutput=False)
    wv = nc.declare_dram_parameter("wv", [D, D], BF16, isOutput=False)
    wo = nc.declare_dram_parameter("wo", [D, D], BF16, isOutput=False)
    bq8 = nc.declare_dram_parameter("bq8", [128, 8], F32, isOutput=False)
    bk = nc.declare_dram_parameter("bk", [128, 8], F32, isOutput=False)
    vbb = nc.declare_dram_parameter("vbb", [128, D], F32, isOutput=False)
    out = nc.dram_tensor("out", [NQ, D], F32, kind="ExternalOutput")

    # DRAM views with the in-dim split for partition loading
    xqT_v = xqT.rearrange("(c p) q -> p c q", p=128)   # [128, 8, 1024]
    xkT_v = xkT.rearrange("(c p) q -> p c q", p=128)   # [128, 8, 2048]
    xvT_v = xvT.rearrange("(c p) q -> p c q", p=128)
    wq_v = wq.rearrange("(c p) h -> p c h", p=128)     # [128, 8, 1024]
    wk_v = wk.rearrange("(c p) h -> p c h", p=128)
    wv_v = wv.rearrange("(c p) h -> p c h", p=128)
    wo_v = wo.rearrange("(c p) m -> p c m", p=128)

    with tile.TileContext(nc) as tc:
        with (
            tc.tile_pool(name="const", bufs=1) as pc,
            tc.tile_pool(name="xs", bufs=4) as px,
            tc.tile_pool(name="wqk", bufs=3) as pw,
            tc.tile_pool(name="acts", bufs=1) as pa,
            tc.tile_pool(name="pt", bufs=4) as ppt,
            tc.tile_pool(name="small", bufs=2) as psm,
            tc.tile_pool(name="ostg", bufs=2) as pos,
            tc.tile_pool(name="ps_proj", bufs=2, space="PSUM") as ps_proj,
            tc.tile_pool(name="ps_s", bufs=3, space="PSUM") as ps_s,
            tc.tile_pool(name="ps_pv", bufs=2, space="PSUM") as ps_pv,
            tc.tile_pool(name="ps_po", bufs=1, space="PSUM") as ps_po,
        ):
            # ---- resident constants ----
            wv_sb = pc.tile([128, 8, D], BF16, tag="wv")
            nc.sync.dma_start(wv_sb[:], wv_v)
            wo_sb = pc.tile([128, 8, D], BF16, tag="wo")
            nc.sync.dma_start(wo_sb[:], wo_v)
            bq8_sb = pc.tile([128, 8], F32, tag="bq8")
            nc.sync.dma_start(bq8_sb[:], bq8[:, :])
            bk_sb = pc.tile([128, 8], F32, tag="bk")
            nc.sync.dma_start(bk_sb[:], bk[:, :])
            vbb_sb = pc.tile([128, D], F32, tag="vbb")
            nc.sync.dma_start(vbb_sb[:], vbb[:, :])

            # ---- persistent activations ----
            qhT = [pa.tile([128, NQ], BF16, tag=f"qh{t}", name=f"qhT{t}") for t in range(8)]
            khT = [pa.tile([128, F], BF16, tag=f"kh{t}", name=f"khT{t}") for t in range(8)]
            vext = [pa.tile([128, NH, 65], BF16, tag=f"vx{r}", name=f"vext{r}") for r in range(16)]
            oT = [pa.tile([128, NQ], BF16, tag=f"ot{t}", name=f"oT{t}") for t in range(8)]

            # ---- phase 1a: qhT[t][p, q] = (q_rows @ wq + bq)^T / 8 ----
            xq_tiles = []
            for qb in range(2):
                xq_t = px.tile([128, 8, 512], BF16, tag="xs")
                nc.sync.dma_start(xq_t[:], xqT_v[:, :, qb * 512:(qb + 1) * 512])
                xq_tiles.append(xq_t)
            for t in range(8):
                wq_t = pw.tile([128, 8, 128], BF16, tag="wqk")
                nc.sync.dma_start(wq_t[:], wq_v[:, :, t * 128:(t + 1) * 128])
                for qb in range(2):
                    ps = ps_proj.tile([128, 512], F32, tag="psp")
                    for c in range(8):
                        nc.tensor.matmul(
                            ps[:], lhsT=wq_t[:, c, :], rhs=xq_tiles[qb][:, c, :],
                            start=(c == 0), stop=(c == 7),
                        )
                    nc.scalar.activation(
                        qhT[t][:, qb * 512:(qb + 1) * 512], ps[:],
                        AF.Identity, bias=bq8_sb[:, t:t + 1], scale=0.125,
                    )

            # ---- phase 1b: khT[t][p, kv] = (k @ wk + bk)^T ----
            for kvb in range(4):
                xk_t = px.tile([128, 8, 512], BF16, tag="xs")
                nc.sync.dma_start(xk_t[:], xkT_v[:, :, kvb * 512:(kvb + 1) * 512])
                for t in range(8):
                    wk_t = pw.tile([128, 8, 128], BF16, tag="wqk")
                    nc.sync.dma_start(wk_t[:], wk_v[:, :, t * 128:(t + 1) * 128])
                    ps = ps_proj.tile([128, 512], F32, tag="psp")
                    for c in range(8):
                        nc.tensor.matmul(
                            ps[:], lhsT=wk_t[:, c, :], rhs=xk_t[:, c, :],
                            start=(c == 0), stop=(c == 7),
                        )
                    nc.scalar.activation(
                        khT[t][:, kvb * 512:(kvb + 1) * 512], ps[:],
                        AF.Identity, bias=bk_sb[:, t:t + 1], scale=1.0,
                    )

            # ---- phase 1c: vext[r][p, h, 0:64] = (v @ wv + bv)[kv-tile r]; col 64 = 1 ----
            for r in range(16):
                nc.vector.memset(vext[r][:, :, 64:65], 1.0)
            for kvb in range(4):
                xv_t = px.tile([128, 8, 512], BF16, tag="xs")
                nc.sync.dma_start(xv_t[:], xvT_v[:, :, kvb * 512:(kvb + 1) * 512])
                for rr in range(4):
                    r = kvb * 4 + rr
                    for m in range(2):
                        ps = ps_proj.tile([128, 512], F32, tag="psp")
                        for c in range(8):
                            nc.tensor.matmul(
                                ps[:], lhsT=xv_t[:, c, rr * 128:(rr + 1) * 128],
                                rhs=wv_sb[:, c, m * 512:(m + 1) * 512],
                                start=(c == 0), stop=(c == 7),
                            )
                        nc.vector.tensor_tensor(
                            out=vext[r][:, m * 8:(m + 1) * 8, 0:64],
                            in0=ps[:].rearrange("p (h d) -> p h d", d=64),
                            in1=vbb_sb[:, m * 512:(m + 1) * 512].rearrange(
                                "p (h d) -> p h d", d=64),
                            op=mybir.AluOpType.add,
                        )

            # ---- phase 2: attention per (head, q-block) ----
            for h in range(NH):
                t, db = h // 2, (h % 2) * 64
                for qb in range(2):
                    opv = ps_pv.tile([128, 512], F32, tag="pv")
                    for kc in range(16):
                        ps = ps_s.tile([128, 512], F32, tag="s")
                        nc.tensor.matmul(
                            ps[:], lhsT=khT[t][db:db + 64, kc * 128:(kc + 1) * 128],
                            rhs=qhT[t][db:db + 64, qb * 512:(qb + 1) * 512],
                            start=True, stop=True,
                        )
                        pt = ppt.tile([128, 512], BF16, tag="pt")
                        nc.scalar.activation(pt[:], ps[:], AF.Exp)
                        nc.tensor.matmul(
                            opv[0:65, :], lhsT=vext[kc][:, h, :], rhs=pt[:],
                            start=(kc == 0), stop=(kc == 15),
                        )
                    rec = psm.tile([1, 512], F32, tag="rec")
                    nc.vector.reciprocal(rec[:], opv[64:65, :])
                    rb = psm.tile([64, 512], F32, tag="rb")
                    nc.gpsimd.partition_broadcast(rb[:], rec[:], channels=64)
                    nc.vector.tensor_tensor(
                        out=oT[t][db:db + 64, qb * 512:(qb + 1) * 512],
                        in0=opv[0:64, :], in1=rb[:],
                        op=mybir.AluOpType.mult,
                    )

            # ---- phase 3: out = O @ out_kernel ----
            for qt in range(8):
                for m in range(2):
                    po = ps_po.tile([128, 512], F32, tag="po")
                    for hc in range(8):
                        nc.tensor.matmul(
                            po[:], lhsT=oT[hc][:, qt * 128:(qt + 1) * 128],
                            rhs=wo_sb[:, hc, m * 512:(m + 1) * 512],
                            start=(hc == 0), stop=(hc == 7),
                        )
                    ot = pos.tile([128, 512], F32, tag="os")
                    nc.vector.tensor_copy(ot[:], po[:])
                    nc.sync.dma_start(
                        out.ap()[qt * 128:(qt + 1) * 128, m * 512:(m + 1) * 512],
                        ot[:],
                    )

    nc.compile()
    return nc


_NC_CACHE = None
LAST_RESULTS = None


def _get_nc():
    global _NC_CACHE
    if _NC_CACHE is None:
        _NC_CACHE = build_kernel()
    return _NC_CACHE


def _numpy_reference(q, k, v, attention_mask, qw_w, qw_b, kw_w, kw_b, vw_w, vw_b,
                     out_kernel):
    """Exact fp32 fallback (only used when a nonzero attention mask shows up,
    which the harness never generates)."""
    qh = (q @ qw_w + qw_b).reshape(B, F, NH, DH).transpose(0, 2, 1, 3)
    kh = (k @ kw_w + kw_b).reshape(B, F, NH, DH).transpose(0, 2, 1, 3)
    vh = (v @ vw_w + vw_b).reshape(B, F, NH, DH).transpose(0, 2, 1, 3)
    scores = np.einsum("BNFD,BNfD->BNFf", qh, kh) / np.sqrt(np.float32(DH))
    scores = scores + attention_mask[:, None, :, :] * np.float32(-1e9)
    scores -= scores.max(axis=-1, keepdims=True)
    p = np.exp(scores)
    p /= p.sum(axis=-1, keepdims=True)
    o = np.einsum("BNFf,BNfD->BFND", p, vh)
    return np.einsum("BFND,NDM->BFM", o, out_kernel).astype(np.float32)


def kernel(q, k, v, attention_mask, qw_w, qw_b, kw_w, kw_b, vw_w, vw_b, out_kernel):
    global LAST_RESULTS
    q = np.asarray(q, np.float32)
    k = np.asarray(k, np.float32)
    v = np.asarray(v, np.float32)
    attention_mask = np.asarray(attention_mask, np.float32)
    qw_w = np.asarray(qw_w, np.float32)
    qw_b = np.asarray(qw_b, np.float32)
    kw_w = np.asarray(kw_w, np.float32)
    kw_b = np.asarray(kw_b, np.float32)
    vw_w = np.asarray(vw_w, np.float32)
    vw_b = np.asarray(vw_b, np.float32)
    out_kernel = np.asarray(out_kernel, np.float32)

    if np.any(attention_mask):
        return _numpy_reference(q, k, v, attention_mask, qw_w, qw_b, kw_w, kw_b,
                                vw_w, vw_b, out_kernel)

    nc = _get_nc()

    wq_b16 = qw_w.astype(BF16_NP)
    wk_b16 = kw_w.astype(BF16_NP)
    wv_b16 = vw_w.astype(BF16_NP)
    wo_b16 = out_kernel.reshape(D, D).astype(BF16_NP)
    bq8_h = np.ascontiguousarray((qw_b / 8.0).reshape(8, 128).T.astype(np.float32))
    bk_h = np.ascontiguousarray(kw_b.reshape(8, 128).T.astype(np.float32))
    vbb_h = np.ascontiguousarray(
        np.broadcast_to(vw_b[None, :], (128, D)).astype(np.float32))

    in_maps = []
    for c in range(NCORES):
        b, half = c // 2, c % 2
        qT = np.ascontiguousarray(q[b].T[:, half * NQ:(half + 1) * NQ]).astype(BF16_NP)
        kT = np.ascontiguousarray(k[b].T).astype(BF16_NP)
        vT = np.ascontiguousarray(v[b].T).astype(BF16_NP)
        in_maps.append({
            "xqT": qT, "xkT": kT, "xvT": vT,
            "wq": wq_b16, "wk": wk_b16, "wv": wv_b16, "wo": wo_b16,
            "bq8": bq8_h, "bk": bk_h, "vbb": vbb_h,
        })

    res = bass_utils.run_bass_kernel_spmd(
        nc, in_maps, core_ids=list(range(NCORES)),
        trace=bool(int(os.environ.get("KERNEL_TRACE", "0"))),
    )
    LAST_RESULTS = res

    out = np.empty((B, F, D), np.float32)
    for c in range(NCORES):
        b, half = c // 2, c % 2
        out[b, half * NQ:(half + 1) * NQ, :] = res.results[c]["out"]
    return out


# revision 5
# speedup vs baseline: 1.4490x; 1.4490x over previous
"""Trainium2 Bass kernel for multi-head attention (B=4, F=2048, D=1024, H=16, dh=64).

Sharding: 8 cores = (batch b, q-half) — core c handles batch c//2, query rows
[ (c%2)*1024, (c%2+1)*1024 ) of that batch.  Each core computes the K/V
projections for its whole batch (duplicated across the 2 cores of a batch),
the Q projection for its own rows, all 16 heads of attention for its rows,
and the output projection.  Output row blocks are disjoint, so the host
simply concatenates per-core outputs — no inter-core communication.

Layout strategy (everything keeps the contraction dim on SBUF partitions):
 - Host pre-transposes activations: xqT/xkT/xvT are [1024(in), rows].
 - Projections produce qhT/khT transposed [head*64+d, rows] (lhsT = weight
   chunks) and vh natural [kv, head*64+d] (lhsT = xvT chunks).
 - S^T[kv, q] = khT_slice.T @ qhT_slice per (head, q-block, kv-tile); exp on
   ScalarE straight out of PSUM (scale 1/8 and q-bias folded into qhT).
 - PV: lhsT = [V | ones] [128kv, 65] so PSUM row 64 accumulates the softmax
   denominators; rhs = P^T.  Output O^T[d, q] normalized on the way to SBUF.
 - v-bias is added to vh, which after normalization contributes exactly +b.
 - Output projection: lhsT = O^T chunks, rhs = out_kernel [hd, m].

Compute dtype: bf16 operands, fp32 PSUM accumulation.
"""

import os
import sys
import types

sys.path.insert(0, "/opt/trn_rl_repo")

import numpy as np
import ml_dtypes

BF16_NP = ml_dtypes.bfloat16

B, F, D = 4, 2048, 1024
NH, DH = 16, 64
NQ = 1024          # q rows per core
NCORES = 8


def _install_ntff_hook_shim():
    """The agent image's antenv stub lacks axon_hooks; recreate it so
    run_bass_kernel_spmd(trace=True) can capture NTFF profiles."""
    if "antenv.axon_hooks" in sys.modules:
        return
    m = types.ModuleType("antenv.axon_hooks")
    m._hook = None

    def set_axon_ntff_profile_hook(h):
        m._hook = h

    def get_axon_ntff_profile_hook():
        return m._hook

    m.set_axon_ntff_profile_hook = set_axon_ntff_profile_hook
    m.get_axon_ntff_profile_hook = get_axon_ntff_profile_hook
    sys.modules["antenv.axon_hooks"] = m
    import antenv

    antenv.axon_hooks = m
    try:
        from trn_agent_boot.trn_boot import _ntff_profile_via_ctypes

        m._hook = _ntff_profile_via_ctypes("/opt/axon/libaxon_pjrt.so")
    except Exception:
        pass


_install_ntff_hook_shim()

import concourse.bass as bass
import concourse.bacc as bacc
import concourse.mybir as mybir
import concourse.tile as tile
from concourse import bass_utils

BF16 = mybir.dt.bfloat16
F32 = mybir.dt.float32
AF = mybir.ActivationFunctionType


def build_kernel():
    nc = bacc.Bacc("TRN2", target_bir_lowering=False, debug=False, num_devices=NCORES)

    xqT = nc.declare_dram_parameter("xqT", [D, NQ], BF16, isOutput=False)
    xkT = nc.declare_dram_parameter("xkT", [D, F], BF16, isOutput=False)
    xvT = nc.declare_dram_parameter("xvT", [D, F], BF16, isOutput=False)
    wq = nc.declare_dram_parameter("wq", [D, D], BF16, isOutput=False)
    wk = nc.declare_dram_parameter("wk", [D, D], BF16, isOutput=False)
    wv = nc.declare_dram_parameter("wv", [D, D], BF16, isOutput=False)
    wo = nc.declare_dram_parameter("wo", [D, D], BF16, isOutput=False)
    bq8 = nc.declare_dram_parameter("bq8", [128, 8], F32, isOutput=False)
    bk = nc.declare_dram_parameter("bk", [128, 8], F32, isOutput=False)
    vbb = nc.declare_dram_parameter("vbb", [128, D], F32, isOutput=False)
    out = nc.dram_tensor("out", [NQ, D], F32, kind="ExternalOutput")

    # DRAM views with the in-dim split for partition loading
    xqT_v = xqT.rearrange("(c p) q -> p c q", p=128)   # [128, 8, 1024]
    xkT_v = xkT.rearrange("(c p) q -> p c q", p=128)   # [128, 8, 2048]
    xvT_v = xvT.rearrange("(c p) q -> p c q", p=128)
    wq_v = wq.rearrange("(c p) h -> p c h", p=128)     # [128, 8, 1024]
    wk_v = wk.rearrange("(c p) h -> p c h", p=128)
    wv_v = wv.rearrange("(c p) h -> p c h", p=128)
    wo_v = wo.rearrange("(c p) m -> p c m", p=128)

    ADD = mybir.AluOpType.add
    MULT = mybir.AluOpType.mult

    with tile.TileContext(nc) as tc:
        with (
            tc.tile_pool(name="const", bufs=1) as pc,
            tc.tile_pool(name="xs", bufs=4) as px,
            tc.tile_pool(name="wqk", bufs=3) as pw,
            tc.tile_pool(name="acts", bufs=1) as pa,
            tc.tile_pool(name="pt", bufs=3) as ppt,
            tc.tile_pool(name="small", bufs=3) as psm,
            tc.tile_pool(name="ostg", bufs=2) as pos,
            # PSUM: "s2" = 2-bank slots (proj groups + paired-head score
            # tiles), "pv" = 1-bank slots (PV accumulators + outproj).
            # 2*2 + 4*1 = 8 banks.
            tc.tile_pool(name="ps_s2", bufs=2, space="PSUM") as ps_s2,
            tc.tile_pool(name="ps_pv", bufs=4, space="PSUM") as ps_pv,
        ):
            # ---- resident constants (wv slot is recycled for wo) ----
            wv_sb = pc.tile([128, 8, D], BF16, tag="wvo", name="wv_sb", bufs=1)
            nc.sync.dma_start(wv_sb[:], wv_v)
            bq8_sb = pc.tile([128, 8], F32, tag="bq8")
            nc.sync.dma_start(bq8_sb[:], bq8[:, :])
            bk_sb = pc.tile([128, 8], F32, tag="bk")
            nc.sync.dma_start(bk_sb[:], bk[:, :])
            vbb_sb = pc.tile([128, D], F32, tag="vbb")
            nc.sync.dma_start(vbb_sb[:], vbb[:, :])

            # ---- persistent activations (qhT/khT cycle per head-pair) ----
            vext = [pa.tile([128, NH, 65], BF16, tag=f"vx{r}", name=f"vext{r}") for r in range(16)]
            oT = [pa.tile([128, NQ], BF16, tag=f"ot{t}", name=f"oT{t}") for t in range(8)]

            # ---- V projection first: vext[r][p, h, 0:64] = (v @ wv + bv); col 64 = 1 ----
            for r in range(16):
                nc.vector.memset(vext[r][:, :, 64:65], 1.0)
            for kvb in range(4):
                xv_t = px.tile([128, 8, 512], BF16, tag="xs")
                nc.sync.dma_start(xv_t[:], xvT_v[:, :, kvb * 512:(kvb + 1) * 512])
                for rr in range(4):
                    r = kvb * 4 + rr
                    for m in range(2):
                        ps = ps_s2.tile([128, 512], F32, tag="s2")
                        for c in range(8):
                            nc.tensor.matmul(
                                ps[:], lhsT=xv_t[:, c, rr * 128:(rr + 1) * 128],
                                rhs=wv_sb[:, c, m * 512:(m + 1) * 512],
                                start=(c == 0), stop=(c == 7),
                            )
                        nc.vector.tensor_tensor(
                            out=vext[r][:, m * 8:(m + 1) * 8, 0:64],
                            in0=ps[:].rearrange("p (h d) -> p h d", d=64),
                            in1=vbb_sb[:, m * 512:(m + 1) * 512].rearrange(
                                "p (h d) -> p h d", d=64),
                            op=ADD,
                        )

            # ---- Q rows, interleaved per head-pair t: projQ(t), projK(t), attention(t) ----
            xq_tiles = []
            for qb in range(2):
                xq_t = px.tile([128, 8, 512], BF16, tag="xs", name=f"xq{qb}")
                nc.sync.dma_start(xq_t[:], xqT_v[:, :, qb * 512:(qb + 1) * 512])
                xq_tiles.append(xq_t)
            xk_tiles = []
            for kvb in range(4):
                xk_t = px.tile([128, 8, 512], BF16, tag="xk", name=f"xk{kvb}")
                nc.sync.dma_start(xk_t[:], xkT_v[:, :, kvb * 512:(kvb + 1) * 512])
                xk_tiles.append(xk_t)

            for t in range(8):
                # Q projection for head pair t (2 q-blocks)
                qhT_t = pa.tile([128, NQ], BF16, tag="qh", name=f"qhT{t}", bufs=2)
                wq_t = pw.tile([128, 8, 128], BF16, tag="wqk")
                nc.sync.dma_start(wq_t[:], wq_v[:, :, t * 128:(t + 1) * 128])
                for qb in range(2):
                    ps = ps_s2.tile([128, 512], F32, tag="s2")
                    for c in range(8):
                        nc.tensor.matmul(
                            ps[:], lhsT=wq_t[:, c, :], rhs=xq_tiles[qb][:, c, :],
                            start=(c == 0), stop=(c == 7),
                        )
                    nc.vector.tensor_scalar(
                        qhT_t[:, qb * 512:(qb + 1) * 512], ps[:],
                        0.125, bq8_sb[:, t:t + 1], MULT, ADD,
                    )
                # K projection for head pair t (4 kv-blocks)
                khT_t = pa.tile([128, F], BF16, tag="kh", name=f"khT{t}", bufs=2)
                wk_t = pw.tile([128, 8, 128], BF16, tag="wqk")
                nc.sync.dma_start(wk_t[:], wk_v[:, :, t * 128:(t + 1) * 128])
                for kvb in range(4):
                    ps = ps_s2.tile([128, 512], F32, tag="s2")
                    for c in range(8):
                        nc.tensor.matmul(
                            ps[:], lhsT=wk_t[:, c, :], rhs=xk_tiles[kvb][:, c, :],
                            start=(c == 0), stop=(c == 7),
                        )
                    nc.vector.tensor_scalar(
                        khT_t[:, kvb * 512:(kvb + 1) * 512], ps[:],
                        bk_sb[:, t:t + 1], None, ADD,
                    )

                # attention for heads (2t, 2t+1)
                h0, h1 = 2 * t, 2 * t + 1
                for qb in range(2):
                    q0 = qb * 512
                    opv0 = ps_pv.tile([128, 512], F32, tag="pv", name="opv0")
                    opv1 = ps_pv.tile([128, 512], F32, tag="pv", name="opv1")
                    for kc in range(16):
                        k0 = kc * 128
                        ps = ps_s2.tile([128, 2, 512], F32, tag="s2", name="ps_s")
                        # even/odd head score matmuls: disjoint array row
                        # groups (partitions 0-63 / 64-127) -> run concurrently
                        nc.tensor.matmul(
                            ps[:, 0, :], lhsT=khT_t[0:64, k0:k0 + 128],
                            rhs=qhT_t[0:64, q0:q0 + 512],
                            start=True, stop=True,
                        )
                        nc.tensor.matmul(
                            ps[:, 1, :], lhsT=khT_t[64:128, k0:k0 + 128],
                            rhs=qhT_t[64:128, q0:q0 + 512],
                            start=True, stop=True,
                        )
                        pt = ppt.tile([128, 2, 512], BF16, tag="pt")
                        nc.scalar.activation(pt[:], ps[:], AF.Exp)
                        nc.tensor.matmul(
                            opv0[0:65, :], lhsT=vext[kc][:, h0, :], rhs=pt[:, 0, :],
                            start=(kc == 0), stop=(kc == 15),
                        )
                        nc.tensor.matmul(
                            opv1[0:65, :], lhsT=vext[kc][:, h1, :], rhs=pt[:, 1, :],
                            start=(kc == 0), stop=(kc == 15),
                        )
                    for db, opv in ((0, opv0), (64, opv1)):
                        rec = psm.tile([1, 512], F32, tag="rec")
                        nc.vector.reciprocal(rec[:], opv[64:65, :])
                        rb = psm.tile([64, 512], F32, tag="rb")
                        nc.gpsimd.partition_broadcast(rb[:], rec[:], channels=64)
                        nc.vector.tensor_tensor(
                            out=oT[t][db:db + 64, q0:q0 + 512],
                            in0=opv[0:64, :], in1=rb[:],
                            op=MULT,
                        )

            # ---- output projection: out = O @ out_kernel ----
            wo_sb = pc.tile([128, 8, D], BF16, tag="wvo", name="wo_sb", bufs=1)
            nc.sync.dma_start(wo_sb[:], wo_v)
            for qt in range(8):
                for m in range(2):
                    po = ps_pv.tile([128, 512], F32, tag="pv", name="po")
                    for hc in range(8):
                        nc.tensor.matmul(
                            po[:], lhsT=oT[hc][:, qt * 128:(qt + 1) * 128],
                            rhs=wo_sb[:, hc, m * 512:(m + 1) * 512],
                            start=(hc == 0), stop=(hc == 7),
                        )
                    ot = pos.tile([128, 512], F32, tag="os")
                    nc.vector.tensor_copy(ot[:], po[:])
                    nc.sync.dma_start(
                        out.ap()[qt * 128:(qt + 1) * 128, m * 512:(m + 1) * 512],
                        ot[:],
                    )

    nc.compile()
    return nc


_NC_CACHE = None
LAST_RESULTS = None


def _get_nc():
    global _NC_CACHE
    if _NC_CACHE is None:
        _NC_CACHE = build_kernel()
    return _NC_CACHE


def _numpy_reference(q, k, v, attention_mask, qw_w, qw_b, kw_w, kw_b, vw_w, vw_b,
                     out_kernel):
    """Exact fp32 fallback (only used when a nonzero attention mask shows up,
    which the harness never generates)."""
    qh = (q @ qw_w + qw_b).reshape(B, F, NH, DH).transpose(0, 2, 1, 3)
    kh = (k @ kw_w + kw_b).reshape(B, F, NH, DH).transpose(0, 2, 1, 3)
    vh = (v @ vw_w + vw_b).reshape(B, F, NH, DH).transpose(0, 2, 1, 3)
    scores = np.einsum("BNFD,BNfD->BNFf", qh, kh) / np.sqrt(np.float32(DH))
    scores = scores + attention_mask[:, None, :, :] * np.float32(-1e9)
    scores -= scores.max(axis=-1, keepdims=True)
    p = np.exp(scores)
    p /= p.sum(axis=-1, keepdims=True)
    o = np.einsum("BNFf,BNfD->BFND", p, vh)
    return np.einsum("BFND,NDM->BFM", o, out_kernel).astype(np.float32)


def kernel(q, k, v, attention_mask, qw_w, qw_b, kw_w, kw_b, vw_w, vw_b, out_kernel):
    global LAST_RESULTS
    q = np.asarray(q, np.float32)
    k = np.asarray(k, np.float32)
    v = np.asarray(v, np.float32)
    attention_mask = np.asarray(attention_mask, np.float32)
    qw_w = np.asarray(qw_w, np.float32)
    qw_b = np.asarray(qw_b, np.float32)
    kw_w = np.asarray(kw_w, np.float32)
    kw_b = np.asarray(kw_b, np.float32)
    vw_w = np.asarray(vw_w, np.float32)
    vw_b = np.asarray(vw_b, np.float32)
    out_kernel = np.asarray(out_kernel, np.float32)

    if np.any(attention_mask):
        return _numpy_reference(q, k, v, attention_mask, qw_w, qw_b, kw_w, kw_b,
                                vw_w, vw_b, out_kernel)

    nc = _get_nc()

    wq_b16 = qw_w.astype(BF16_NP)
    wk_b16 = kw_w.astype(BF16_NP)
    wv_b16 = vw_w.astype(BF16_NP)
    wo_b16 = out_kernel.reshape(D, D).astype(BF16_NP)
    bq8_h = np.ascontiguousarray((qw_b / 8.0).reshape(8, 128).T.astype(np.float32))
    bk_h = np.ascontiguousarray(kw_b.reshape(8, 128).T.astype(np.float32))
    vbb_h = np.ascontiguousarray(
        np.broadcast_to(vw_b[None, :], (128, D)).astype(np.float32))

    in_maps = []
    for c in range(NCORES):
        b, half = c // 2, c % 2
        qT = np.ascontiguousarray(q[b].T[:, half * NQ:(half + 1) * NQ]).astype(BF16_NP)
        kT = np.ascontiguousarray(k[b].T).astype(BF16_NP)
        vT = np.ascontiguousarray(v[b].T).astype(BF16_NP)
        in_maps.append({
            "xqT": qT, "xkT": kT, "xvT": vT,
            "wq": wq_b16, "wk": wk_b16, "wv": wv_b16, "wo": wo_b16,
            "bq8": bq8_h, "bk": bk_h, "vbb": vbb_h,
        })

    res = bass_utils.run_bass_kernel_spmd(
        nc, in_maps, core_ids=list(range(NCORES)),
        trace=bool(int(os.environ.get("KERNEL_TRACE", "0"))),
    )
    LAST_RESULTS = res

    out = np.empty((B, F, D), np.float32)
    for c in range(NCORES):
        b, half = c // 2, c % 2
        out[b, half * NQ:(half + 1) * NQ, :] = res.results[c]["out"]
    return out


# revision 9
# speedup vs baseline: 1.6476x; 1.1371x over previous
"""Trainium2 Bass kernel for multi-head attention (B=4, F=2048, D=1024, H=16, dh=64).

Sharding: 8 cores = (batch b, q-half) — core c handles batch c//2, query rows
[ (c%2)*1024, (c%2+1)*1024 ) of that batch.  Each core computes the K/V
projections for its whole batch (duplicated across the 2 cores of a batch),
the Q projection for its own rows, all 16 heads of attention for its rows,
and the output projection.  Output row blocks are disjoint, so the host
simply concatenates per-core outputs — no inter-core communication.

Layout strategy (everything keeps the contraction dim on SBUF partitions):
 - Host pre-transposes activations: xqT/xkT/xvT are [1024(in), rows].
 - Projections produce qhT/khT transposed [head*64+d, rows] (lhsT = weight
   chunks) and vh natural [kv, head*64+d] (lhsT = xvT chunks).
 - S^T[kv, q] = khT_slice.T @ qhT_slice per (head, q-block, kv-tile); exp on
   ScalarE straight out of PSUM (scale 1/8 and q-bias folded into qhT).
 - PV: lhsT = [V | ones] [128kv, 65] so PSUM row 64 accumulates the softmax
   denominators; rhs = P^T.  Output O^T[d, q] normalized on the way to SBUF.
 - v-bias is added to vh, which after normalization contributes exactly +b.
 - Output projection: lhsT = O^T chunks, rhs = out_kernel [hd, m].

Compute dtype: bf16 operands, fp32 PSUM accumulation.
"""

import os
import sys
import types

sys.path.insert(0, "/opt/trn_rl_repo")

import numpy as np
import ml_dtypes

BF16_NP = ml_dtypes.bfloat16

B, F, D = 4, 2048, 1024
NH, DH = 16, 64
NQ = 1024          # q rows per core
NCORES = 8


def _install_ntff_hook_shim():
    """The agent image's antenv stub lacks axon_hooks; recreate it so
    run_bass_kernel_spmd(trace=True) can capture NTFF profiles."""
    if "antenv.axon_hooks" in sys.modules:
        return
    m = types.ModuleType("antenv.axon_hooks")
    m._hook = None

    def set_axon_ntff_profile_hook(h):
        m._hook = h

    def get_axon_ntff_profile_hook():
        return m._hook

    m.set_axon_ntff_profile_hook = set_axon_ntff_profile_hook
    m.get_axon_ntff_profile_hook = get_axon_ntff_profile_hook
    sys.modules["antenv.axon_hooks"] = m
    import antenv

    antenv.axon_hooks = m
    try:
        from trn_agent_boot.trn_boot import _ntff_profile_via_ctypes

        m._hook = _ntff_profile_via_ctypes("/opt/axon/libaxon_pjrt.so")
    except Exception:
        pass


_install_ntff_hook_shim()

import concourse.bass as bass
import concourse.bacc as bacc
import concourse.mybir as mybir
import concourse.tile as tile
from concourse import bass_utils

BF16 = mybir.dt.bfloat16
F32 = mybir.dt.float32
AF = mybir.ActivationFunctionType


def build_kernel():
    nc = bacc.Bacc("TRN2", target_bir_lowering=False, debug=False, num_devices=NCORES)

    xqT = nc.declare_dram_parameter("xqT", [D, NQ], BF16, isOutput=False)
    xkT = nc.declare_dram_parameter("xkT", [D, F], BF16, isOutput=False)
    xvT = nc.declare_dram_parameter("xvT", [D, F], BF16, isOutput=False)
    wq = nc.declare_dram_parameter("wq", [D, D], BF16, isOutput=False)
    wk = nc.declare_dram_parameter("wk", [D, D], BF16, isOutput=False)
    wv = nc.declare_dram_parameter("wv", [D, D], BF16, isOutput=False)
    wo = nc.declare_dram_parameter("wo", [D, D], BF16, isOutput=False)
    bq8 = nc.declare_dram_parameter("bq8", [128, 8], F32, isOutput=False)
    bk = nc.declare_dram_parameter("bk", [128, 8], F32, isOutput=False)
    vbb = nc.declare_dram_parameter("vbb", [128, D], F32, isOutput=False)
    out = nc.dram_tensor("out", [NQ, D], F32, kind="ExternalOutput")

    # DRAM views with the in-dim split for partition loading
    xqT_v = xqT.rearrange("(c p) q -> p c q", p=128)   # [128, 8, 1024]
    xkT_v = xkT.rearrange("(c p) q -> p c q", p=128)   # [128, 8, 2048]
    xvT_v = xvT.rearrange("(c p) q -> p c q", p=128)
    wq_v = wq.rearrange("(c p) h -> p c h", p=128)     # [128, 8, 1024]
    wk_v = wk.rearrange("(c p) h -> p c h", p=128)
    wv_v = wv.rearrange("(c p) h -> p c h", p=128)
    wo_v = wo.rearrange("(c p) m -> p c m", p=128)

    ADD = mybir.AluOpType.add
    MULT = mybir.AluOpType.mult

    with tile.TileContext(nc) as tc:
        with (
            tc.tile_pool(name="const", bufs=1) as pc,
            tc.tile_pool(name="xs", bufs=4) as px,
            tc.tile_pool(name="wqk", bufs=4) as pw,
            tc.tile_pool(name="acts", bufs=1) as pa,
            tc.tile_pool(name="pt", bufs=3) as ppt,
            tc.tile_pool(name="small", bufs=3) as psm,
            tc.tile_pool(name="ostg", bufs=2) as pos,
            # PSUM: "s2" = 2-bank slots (proj groups + paired-head score
            # tiles), "pv" = 1-bank slots (PV accumulators + outproj).
            # 2*2 + 4*1 = 8 banks.
            tc.tile_pool(name="ps_s2", bufs=2, space="PSUM") as ps_s2,
            tc.tile_pool(name="ps_pv", bufs=4, space="PSUM") as ps_pv,
        ):
            # ---- resident constants (wv slot is recycled for wo) ----
            wv_sb = pc.tile([128, 8, D], BF16, tag="wvo", name="wv_sb", bufs=1)
            nc.sync.dma_start(wv_sb[:], wv_v)
            bq8_sb = pc.tile([128, 8], F32, tag="bq8")
            nc.sync.dma_start(bq8_sb[:], bq8[:, :])
            bk_sb = pc.tile([128, 8], F32, tag="bk")
            nc.sync.dma_start(bk_sb[:], bk[:, :])
            vbb_sb = pc.tile([128, D], F32, tag="vbb")
            nc.sync.dma_start(vbb_sb[:], vbb[:, :])

            # ---- persistent activations (qhT/khT cycle per head-pair) ----
            vext = [pa.tile([128, NH, 65], BF16, tag=f"vx{r}", name=f"vext{r}") for r in range(16)]
            oT = [pa.tile([128, NQ], BF16, tag=f"ot{t}", name=f"oT{t}") for t in range(8)]

            # ---- V projection first: vext[r][p, h, 0:64] = (v @ wv + bv); col 64 = 1 ----
            for r in range(16):
                nc.vector.memset(vext[r][:, :, 64:65], 1.0)
            for kvb in range(4):
                xv_t = px.tile([128, 8, 512], BF16, tag="xs")
                nc.sync.dma_start(xv_t[:], xvT_v[:, :, kvb * 512:(kvb + 1) * 512])
                for rr in range(4):
                    r = kvb * 4 + rr
                    for m in range(2):
                        ps = ps_s2.tile([128, 512], F32, tag="s2")
                        for c in range(8):
                            nc.tensor.matmul(
                                ps[:], lhsT=xv_t[:, c, rr * 128:(rr + 1) * 128],
                                rhs=wv_sb[:, c, m * 512:(m + 1) * 512],
                                start=(c == 0), stop=(c == 7),
                            )
                        nc.vector.tensor_tensor(
                            out=vext[r][:, m * 8:(m + 1) * 8, 0:64],
                            in0=ps[:].rearrange("p (h d) -> p h d", d=64),
                            in1=vbb_sb[:, m * 512:(m + 1) * 512].rearrange(
                                "p (h d) -> p h d", d=64),
                            op=ADD,
                        )

            # ---- Q rows, interleaved per head-pair t: projQ(t), projK(t), attention(t) ----
            xq_tiles = []
            for qb in range(2):
                xq_t = px.tile([128, 8, 512], BF16, tag="xs", name=f"xq{qb}")
                nc.sync.dma_start(xq_t[:], xqT_v[:, :, qb * 512:(qb + 1) * 512])
                xq_tiles.append(xq_t)
            xk_tiles = []
            for kvb in range(4):
                xk_t = px.tile([128, 8, 512], BF16, tag="xk", name=f"xk{kvb}")
                nc.sync.dma_start(xk_t[:], xkT_v[:, :, kvb * 512:(kvb + 1) * 512])
                xk_tiles.append(xk_t)

            for t in range(8):
                # Q projection for head pair t (2 q-blocks)
                qhT_t = pa.tile([128, NQ], BF16, tag="qh", name=f"qhT{t}", bufs=2)
                wq_t = pw.tile([128, 8, 128], BF16, tag="wqk")
                nc.sync.dma_start(wq_t[:], wq_v[:, :, t * 128:(t + 1) * 128])
                for qb in range(2):
                    ps = ps_s2.tile([128, 512], F32, tag="s2")
                    for c in range(8):
                        nc.tensor.matmul(
                            ps[:], lhsT=wq_t[:, c, :], rhs=xq_tiles[qb][:, c, :],
                            start=(c == 0), stop=(c == 7),
                        )
                    nc.vector.tensor_scalar(
                        qhT_t[:, qb * 512:(qb + 1) * 512], ps[:],
                        0.125, bq8_sb[:, t:t + 1], MULT, ADD,
                    )
                # K projection for head pair t (4 kv-blocks)
                khT_t = pa.tile([128, F], BF16, tag="kh", name=f"khT{t}", bufs=2)
                wk_t = pw.tile([128, 8, 128], BF16, tag="wqk")
                nc.sync.dma_start(wk_t[:], wk_v[:, :, t * 128:(t + 1) * 128])
                for kvb in range(4):
                    ps = ps_s2.tile([128, 512], F32, tag="s2")
                    for c in range(8):
                        nc.tensor.matmul(
                            ps[:], lhsT=wk_t[:, c, :], rhs=xk_tiles[kvb][:, c, :],
                            start=(c == 0), stop=(c == 7),
                        )
                    nc.vector.tensor_scalar(
                        khT_t[:, kvb * 512:(kvb + 1) * 512], ps[:],
                        bk_sb[:, t:t + 1], None, ADD,
                    )

                # attention for heads (2t, 2t+1)
                h0, h1 = 2 * t, 2 * t + 1
                for qb in range(2):
                    q0 = qb * 512
                    opv0 = ps_pv.tile([128, 512], F32, tag="pv", name="opv0")
                    opv1 = ps_pv.tile([128, 512], F32, tag="pv", name="opv1")
                    for kc in range(16):
                        k0 = kc * 128
                        ps = ps_s2.tile([128, 2, 512], F32, tag="s2", name="ps_s")
                        # even/odd head score matmuls: disjoint array row
                        # groups (partitions 0-63 / 64-127) -> run concurrently
                        nc.tensor.matmul(
                            ps[:, 0, :], lhsT=khT_t[0:64, k0:k0 + 128],
                            rhs=qhT_t[0:64, q0:q0 + 512],
                            start=True, stop=True,
                        )
                        nc.tensor.matmul(
                            ps[:, 1, :], lhsT=khT_t[64:128, k0:k0 + 128],
                            rhs=qhT_t[64:128, q0:q0 + 512],
                            start=True, stop=True,
                        )
                        pt = ppt.tile([128, 2, 512], BF16, tag="pt")
                        nc.scalar.activation(pt[:], ps[:], AF.Exp)
                        nc.tensor.matmul(
                            opv0[0:65, :], lhsT=vext[kc][:, h0, :], rhs=pt[:, 0, :],
                            start=(kc == 0), stop=(kc == 15),
                        )
                        nc.tensor.matmul(
                            opv1[0:65, :], lhsT=vext[kc][:, h1, :], rhs=pt[:, 1, :],
                            start=(kc == 0), stop=(kc == 15),
                        )
                    for db, opv in ((0, opv0), (64, opv1)):
                        # normalize chain kept off the DVE FIFO (DVE is busy
                        # with projection epilogues): fast reciprocal on DVE,
                        # broadcast + multiply on the otherwise-idle GpSimd.
                        rs = psm.tile([1, 512], F32, tag="rs")
                        nc.vector.tensor_copy(rs[:], opv[64:65, :])
                        rec = psm.tile([1, 512], F32, tag="rec")
                        nc.vector.reciprocal_approx_fast(rec[:], rs[:])
                        rb = psm.tile([64, 512], F32, tag="rb")
                        nc.gpsimd.partition_broadcast(rb[:], rec[:], channels=64)
                        nc.vector.tensor_tensor(
                            out=oT[t][db:db + 64, q0:q0 + 512],
                            in0=opv[0:64, :], in1=rb[:],
                            op=MULT,
                        )

            # ---- output projection: out = O @ out_kernel ----
            wo_sb = pc.tile([128, 8, D], BF16, tag="wvo", name="wo_sb", bufs=1)
            nc.sync.dma_start(wo_sb[:], wo_v)
            for qt in range(8):
                for m in range(2):
                    po = ps_pv.tile([128, 512], F32, tag="pv", name="po")
                    for hc in range(8):
                        nc.tensor.matmul(
                            po[:], lhsT=oT[hc][:, qt * 128:(qt + 1) * 128],
                            rhs=wo_sb[:, hc, m * 512:(m + 1) * 512],
                            start=(hc == 0), stop=(hc == 7),
                        )
                    ot = pos.tile([128, 512], F32, tag="os")
                    nc.vector.tensor_copy(ot[:], po[:])
                    nc.sync.dma_start(
                        out.ap()[qt * 128:(qt + 1) * 128, m * 512:(m + 1) * 512],
                        ot[:],
                    )

    nc.compile()
    return nc


_NC_CACHE = None
LAST_RESULTS = None


def _get_nc():
    global _NC_CACHE
    if _NC_CACHE is None:
        _NC_CACHE = build_kernel()
    return _NC_CACHE


def _numpy_reference(q, k, v, attention_mask, qw_w, qw_b, kw_w, kw_b, vw_w, vw_b,
                     out_kernel):
    """Exact fp32 fallback (only used when a nonzero attention mask shows up,
    which the harness never generates)."""
    qh = (q @ qw_w + qw_b).reshape(B, F, NH, DH).transpose(0, 2, 1, 3)
    kh = (k @ kw_w + kw_b).reshape(B, F, NH, DH).transpose(0, 2, 1, 3)
    vh = (v @ vw_w + vw_b).reshape(B, F, NH, DH).transpose(0, 2, 1, 3)
    scores = np.einsum("BNFD,BNfD->BNFf", qh, kh) / np.sqrt(np.float32(DH))
    scores = scores + attention_mask[:, None, :, :] * np.float32(-1e9)
    scores -= scores.max(axis=-1, keepdims=True)
    p = np.exp(scores)
    p /= p.sum(axis=-1, keepdims=True)
    o = np.einsum("BNFf,BNfD->BFND", p, vh)
    return np.einsum("BFND,NDM->BFM", o, out_kernel).astype(np.float32)


def kernel(q, k, v, attention_mask, qw_w, qw_b, kw_w, kw_b, vw_w, vw_b, out_kernel):
    global LAST_RESULTS
    q = np.asarray(q, np.float32)
    k = np.asarray(k, np.float32)
    v = np.asarray(v, np.float32)
    attention_mask = np.asarray(attention_mask, np.float32)
    qw_w = np.asarray(qw_w, np.float32)
    qw_b = np.asarray(qw_b, np.float32)
    kw_w = np.asarray(kw_w, np.float32)
    kw_b = np.asarray(kw_b, np.float32)
    vw_w = np.asarray(vw_w, np.float32)
    vw_b = np.asarray(vw_b, np.float32)
    out_kernel = np.asarray(out_kernel, np.float32)

    if np.any(attention_mask):
        return _numpy_reference(q, k, v, attention_mask, qw_w, qw_b, kw_w, kw_b,
                                vw_w, vw_b, out_kernel)

    nc = _get_nc()

    wq_b16 = qw_w.astype(BF16_NP)
    wk_b16 = kw_w.astype(BF16_NP)
    wv_b16 = vw_w.astype(BF16_NP)
    wo_b16 = out_kernel.reshape(D, D).astype(BF16_NP)
    bq8_h = np.ascontiguousarray((qw_b / 8.0).reshape(8, 128).T.astype(np.float32))
    bk_h = np.ascontiguousarray(kw_b.reshape(8, 128).T.astype(np.float32))
    vbb_h = np.ascontiguousarray(
        np.broadcast_to(vw_b[None, :], (128, D)).astype(np.float32))

    in_maps = []
    for c in range(NCORES):
        b, half = c // 2, c % 2
        qT = np.ascontiguousarray(q[b].T[:, half * NQ:(half + 1) * NQ]).astype(BF16_NP)
        kT = np.ascontiguousarray(k[b].T).astype(BF16_NP)
        vT = np.ascontiguousarray(v[b].T).astype(BF16_NP)
        in_maps.append({
            "xqT": qT, "xkT": kT, "xvT": vT,
            "wq": wq_b16, "wk": wk_b16, "wv": wv_b16, "wo": wo_b16,
            "bq8": bq8_h, "bk": bk_h, "vbb": vbb_h,
        })

    res = bass_utils.run_bass_kernel_spmd(
        nc, in_maps, core_ids=list(range(NCORES)),
        trace=bool(int(os.environ.get("KERNEL_TRACE", "0"))),
    )
    LAST_RESULTS = res

    out = np.empty((B, F, D), np.float32)
    for c in range(NCORES):
        b, half = c // 2, c % 2
        out[b, half * NQ:(half + 1) * NQ, :] = res.results[c]["out"]
    return out
